# revision 2
# baseline (speedup 1.0000x reference)
"""Trainium2 Bass kernel for nn_ContrastiveLoss (NT-Xent style contrastive loss).

Strategy (8 NeuronCores, SPMD):
  - Host sorts samples by label (the scalar loss is permutation invariant),
    row-normalizes, and builds X^T [D=128, N=8192] in bf16.
  - Rows are sharded across 8 cores (1024 rows each, 8 blocks of 128).
  - Each core computes its [1024, 8192] similarity block against the full
    X^T (the "all-gathered" copy arrives as a per-core input), reduces
    exp-row-sums on-chip, and evaluates the positive-pair terms only on a
    narrow label-band window (sorted labels make positives contiguous).
  - Per-row partial losses return to the host, which sums them and divides
    by the exact positive-pair count (from the label histogram).

Math: with e_ij = exp(sim_ij/T), S_i = sum_j e_ij (incl diag),
P_i = sum_{j in label-range(i)} e_ij (incl diag), unsim_i = S_i - P_i,
u_i = log(unsim_i), the reference loss row-sum equals
  npos_i*u_i + sum_{range} softplus(sim_ij/T - u_i) - softplus(1/T - u_i)
             - (sum_{range} sim_ij/T - 1/T)
where npos_i = (label count of i) - 1. The diagonal contributions cancel
exactly in unsim and are removed via the constant sim_ii = 1 (rows are
normalized; the fp difference is ~1e-9 relative on the final scalar).

Perf structure (v2):
  - One activation-table set (natural_log_exp_and_others) serves both Exp
    and Ln, so the ACT engine never reloads tables mid-stream regardless
    of how the tile scheduler interleaves the two.
  - Fully per-block pipeline: each block's band/log tail overlaps the next
    block's dense Exp stream, keeping ACT (the bottleneck engine) busy.
  - Dense Exp runs in-place on PSUM with accum_out row-sums; the exp
    values are never copied to SBUF.
  - Tiny Ln ops (u = ln(unsim), spd = softplus diag) are batched into one
    [128, 16] Ln at the end.
  - Masked band reductions use scalar_tensor_tensor accum_out (one DVE op
    instead of multiply + reduce).
"""

import numpy as np

T = 0.2
INV_T = 1.0 / T  # 5.0
EPS = 1e-5
N, D, NCLASS = 8192, 128, 128
NCORES = 8
ROWS_PER_CORE = N // NCORES          # 1024
BLOCKS = ROWS_PER_CORE // 128        # 8 blocks of 128 rows per core
CHUNK = 2048                         # ACT chunk (4 PSUM banks)
NCHUNKS = N // CHUNK                 # 4 per block
MM = 512                             # matmul free-dim per PSUM bank

_CACHE = {}


def _force_single_act_table_set():
    """Make every activation resolve to natural_log_exp_and_others (which
    contains both Exp and Ln) so the ACT engine loads its spline tables
    exactly once. get_activation_tables is functools.cache'd and returns
    the same dict object to all consumers (bacc's insert_act_table_loads,
    the scheduler cost model), so in-place mutation is seen everywhere.
    Set ids stay valid: keys/order are untouched, other sets just become
    empty and thus never get selected."""
    try:
        from concourse.hw_specs import get_activation_tables
        tabs = get_activation_tables("TRN2")
        if "natural_log_exp_and_others" not in tabs:
            return
        for name in list(tabs.keys()):
            if name != "natural_log_exp_and_others":
                tabs[name] = set()
    except Exception:
        pass  # fall back to default (correct, just slower) table handling


def _build_nc(W, debug=False):
    """Build the SPMD Bass/Tile program. W = band window width (mult of 256)."""
    import concourse.bass as bass
    import concourse.bacc as bacc
    import concourse.mybir as mybir
    import concourse.tile as tile

    _force_single_act_table_set()

    dt = mybir.dt
    AF = mybir.ActivationFunctionType
    ALU = mybir.AluOpType
    X = mybir.AxisListType.X

    nc = bacc.Bacc("TRN2", target_bir_lowering=False, debug=debug)

    xt_d = nc.dram_tensor("xt", [128, N], dt.bfloat16, kind="ExternalInput")
    xtown_d = nc.dram_tensor("xtown", [128, ROWS_PER_CORE], dt.bfloat16,
                             kind="ExternalInput")
    xtband_d = nc.dram_tensor("xtband", [128, BLOCKS * W], dt.bfloat16,
                              kind="ExternalInput")
    gsr_d = nc.dram_tensor("gsr", [128, BLOCKS], dt.float32, kind="ExternalInput")
    ger_d = nc.dram_tensor("ger", [128, BLOCKS], dt.float32, kind="ExternalInput")
    npos_d = nc.dram_tensor("npos", [128, BLOCKS], dt.float32, kind="ExternalInput")
    out_d = nc.dram_tensor("out", [128, BLOCKS], dt.float32, kind="ExternalOutput")

    E5 = float(np.exp(INV_T))

    with tile.TileContext(nc) as tc:
        with (
            tc.tile_pool(name="const", bufs=1) as const,
            tc.tile_pool(name="band", bufs=1) as band,
            tc.tile_pool(name="scr", bufs=3) as scr_pool,
            tc.tile_pool(name="small", bufs=1) as small,
            tc.tile_pool(name="psum", bufs=2, space="PSUM") as psum,
        ):
            # ---- persistent loads (first-needed first) ----
            xtown = const.tile([128, ROWS_PER_CORE], dt.bfloat16)
            nc.sync.dma_start(xtown[:], xtown_d[:])
            xt = const.tile([128, N], dt.bfloat16)
            for k in range(N // CHUNK):
                nc.sync.dma_start(xt[:, k * CHUNK:(k + 1) * CHUNK],
                                  xt_d[:, k * CHUNK:(k + 1) * CHUNK])
            xtband = const.tile([128, BLOCKS * W], dt.bfloat16)
            nc.sync.dma_start(xtband[:], xtband_d[:])
            gsr = const.tile([128, BLOCKS], dt.float32)
            nc.sync.dma_start(gsr[:], gsr_d[:])
            ger = const.tile([128, BLOCKS], dt.float32)
            nc.sync.dma_start(ger[:], ger_d[:])
            npos = const.tile([128, BLOCKS], dt.float32)
            nc.sync.dma_start(npos[:], npos_d[:])

            iota_i = const.tile([128, W], dt.int32)
            nc.gpsimd.iota(iota_i[:], pattern=[[1, W]], base=0, channel_multiplier=0)
            iota_f = const.tile([128, W], dt.float32)
            nc.vector.tensor_copy(iota_f[:], iota_i[:])

            # per-block persistent tiles
            mask = [band.tile([128, W], dt.float32, name=f"mk{b}") for b in range(BLOCKS)]
            e_band = [band.tile([128, W], dt.float32, name=f"eb{b}") for b in range(BLOCKS)]

            # small per-row accumulators, one column per block
            S8 = small.tile([128, BLOCKS], dt.float32, name="S8")
            P8 = small.tile([128, BLOCKS], dt.float32, name="P8")
            A8 = small.tile([128, BLOCKS], dt.float32, name="A8")
            B8 = small.tile([128, BLOCKS], dt.float32, name="B8")
            runsim8 = small.tile([128, BLOCKS], dt.float32, name="rs8")
            # v16 cols 0..7 = unsim per block; cols 8..15 = 1 + E5*runsim
            v16 = small.tile([128, 2 * BLOCKS], dt.float32, name="v16")
            ln16 = small.tile([128, 2 * BLOCKS], dt.float32, name="ln16")
            acc = small.tile([128, BLOCKS], dt.float32, name="acc")

            # ---- masks upfront (DVE only; runs during DMA/matmul fill) ----
            for b in range(BLOCKS):
                m1 = scr_pool.tile([128, W], dt.float32, tag="m1")
                nc.vector.tensor_scalar(m1[:], iota_f[:], gsr[:, b:b + 1], None,
                                        op0=ALU.is_ge)
                nc.vector.scalar_tensor_tensor(mask[b][:], iota_f[:],
                                               ger[:, b:b + 1], m1[:],
                                               op0=ALU.is_lt, op1=ALU.mult)

            # ---- per-block pipeline ----
            for b in range(BLOCKS):
                lhsT = xtown[:, b * 128:(b + 1) * 128]
                sparts = scr_pool.tile([128, NCHUNKS], dt.float32, tag="sparts")
                for kc in range(NCHUNKS):
                    ps = psum.tile([128, CHUNK], dt.float32, tag="ps")
                    for j in range(CHUNK // MM):
                        c0 = kc * CHUNK + j * MM
                        nc.tensor.matmul(ps[:, j * MM:(j + 1) * MM], lhsT,
                                         xt[:, c0:c0 + MM], start=True, stop=True)
                    # Exp in place on PSUM; only the row-sum survives.
                    nc.scalar.activation(ps[:], ps[:], AF.Exp, bias=0.0,
                                         scale=INV_T,
                                         accum_out=sparts[:, kc:kc + 1])
                # band: sims for the W-wide positive window
                psb = psum.tile([128, W], dt.float32, tag="ps")
                nc.tensor.matmul(psb[:, :W], lhsT,
                                 xtband[:, b * W:(b + 1) * W],
                                 start=True, stop=True)
                nc.scalar.activation(e_band[b][:], psb[:, :W], AF.Exp, bias=0.0,
                                     scale=INV_T)
                # B = sum_range sim (read raw sims straight from PSUM)
                scrB = scr_pool.tile([128, W], dt.float32, tag="scrB")
                nc.vector.scalar_tensor_tensor(scrB[:], psb[:, :W], 0.0,
                                               mask[b][:], op0=ALU.bypass,
                                               op1=ALU.mult,
                                               accum_out=B8[:, b:b + 1])
                # S, P, unsim, runsim, 1 + E5*runsim
                nc.vector.reduce_sum(S8[:, b:b + 1], sparts[:], axis=X)
                scrP = scr_pool.tile([128, W], dt.float32, tag="scrP")
                nc.vector.scalar_tensor_tensor(scrP[:], e_band[b][:], 0.0,
                                               mask[b][:], op0=ALU.bypass,
                                               op1=ALU.mult,
                                               accum_out=P8[:, b:b + 1])
                nc.vector.tensor_sub(v16[:, b:b + 1], S8[:, b:b + 1],
                                     P8[:, b:b + 1])
                nc.vector.reciprocal(runsim8[:, b:b + 1], v16[:, b:b + 1])
                nc.vector.tensor_scalar(v16[:, BLOCKS + b:BLOCKS + b + 1],
                                        runsim8[:, b:b + 1], E5, 1.0,
                                        op0=ALU.mult, op1=ALU.add)
                # softplus stream: sp = Ln(1 + e_band*runsim), A = masked sum
                nc.vector.tensor_scalar(e_band[b][:], e_band[b][:],
                                        runsim8[:, b:b + 1], None, op0=ALU.mult)
                sp = scr_pool.tile([128, W], dt.float32, tag="sp")
                nc.scalar.activation(sp[:], e_band[b][:], AF.Ln, bias=1.0)
                scrA = scr_pool.tile([128, W], dt.float32, tag="scrA")
                nc.vector.scalar_tensor_tensor(scrA[:], sp[:], 0.0, mask[b][:],
                                               op0=ALU.bypass, op1=ALU.mult,
                                               accum_out=A8[:, b:b + 1])

            # ---- batched logs + final combine ----
            nc.scalar.activation(ln16[:], v16[:], AF.Ln)  # u8 | ln(1+E5*r)
            u8 = ln16[:, 0:BLOCKS]
            spd8 = ln16[:, BLOCKS:2 * BLOCKS]
            ta = small.tile([128, BLOCKS], dt.float32, name="ta")
            nc.vector.tensor_mul(ta[:], npos[:], u8)
            tb = small.tile([128, BLOCKS], dt.float32, name="tb")
            nc.vector.tensor_add(tb[:], ta[:], A8[:])
            r2 = small.tile([128, BLOCKS], dt.float32, name="r2")
            nc.vector.tensor_scalar(r2[:], B8[:], INV_T, -INV_T,
                                    op0=ALU.mult, op1=ALU.add)
            r3 = small.tile([128, BLOCKS], dt.float32, name="r3")
            nc.vector.tensor_add(r3[:], r2[:], spd8)
            nc.vector.tensor_sub(acc[:], tb[:], r3[:])

            nc.sync.dma_start(out_d[:], acc[:])

    nc.compile()
    return nc


def _prep(input, label):
    """Host-side shard prep: sort by label, normalize, build per-core inputs."""
    import ml_dtypes

    x = np.asarray(input, dtype=np.float32).reshape(N, D)
    lab = np.asarray(label).astype(np.int64).reshape(N)

    order = np.argsort(lab, kind="stable")
    xs, ls = x[order], lab[order]
    counts = np.bincount(ls, minlength=NCLASS)
    n_pos = int((counts.astype(np.int64) ** 2).sum()) - N
    ends = np.cumsum(counts)
    starts = ends - counts
    row_gs = starts[ls]          # [N] group start col per (sorted) row
    row_ge = ends[ls]            # [N] group end col per row

    norms = np.sqrt((xs * xs).sum(1, dtype=np.float32)).astype(np.float32)
    # reference divides by max(n_i*n_j, EPS); for this data the max never
    # binds (norms ~ 11), so plain normalization is exact.
    assert float(norms.min()) ** 2 > EPS * 1.0001
    xn = (xs / norms[:, None]).astype(np.float32)
    xt = np.ascontiguousarray(xn.T).astype(ml_dtypes.bfloat16)  # [128, N]

    # band windows per global block
    nblk = N // 128
    lo = row_gs[np.arange(nblk) * 128]
    hi = row_ge[np.arange(nblk) * 128 + 127]
    maxband = int((hi - lo).max())
    W = max(256, ((maxband + 255) // 256) * 256)
    wstart = np.minimum(lo, N - W)

    in_maps = []
    for c in range(NCORES):
        r0 = c * ROWS_PER_CORE
        xtband = np.empty((128, BLOCKS * W), dtype=ml_dtypes.bfloat16)
        gsr = np.empty((128, BLOCKS), np.float32)
        ger = np.empty((128, BLOCKS), np.float32)
        npos = np.empty((128, BLOCKS), np.float32)
        for b in range(BLOCKS):
            g = c * BLOCKS + b
            ws = int(wstart[g])
            xtband[:, b * W:(b + 1) * W] = xt[:, ws:ws + W]
            rows = slice(r0 + b * 128, r0 + (b + 1) * 128)
            gsr[:, b] = (row_gs[rows] - ws).astype(np.float32)
            ger[:, b] = (row_ge[rows] - ws).astype(np.float32)
            npos[:, b] = (row_ge[rows] - row_gs[rows] - 1).astype(np.float32)
        in_maps.append({
            "xt": xt,
            "xtown": np.ascontiguousarray(
                xt[:, r0:r0 + ROWS_PER_CORE]),
            "xtband": xtband,
            "gsr": gsr,
            "ger": ger,
            "npos": npos,
        })
    return in_maps, n_pos, W


def kernel(input, label):
    from concourse.bass_utils import run_bass_kernel_spmd

    in_maps, n_pos, W = _prep(input, label)
    if W not in _CACHE:
        _CACHE[W] = _build_nc(W)
    nc = _CACHE[W]

    res = None
    for attempt in range(4):
        try:
            res = run_bass_kernel_spmd(nc, in_maps, core_ids=list(range(NCORES)))
            break
        except Exception:
            if attempt == 3:
                raise
            import time
            time.sleep(45)  # device may need a moment to recover
    global LAST_RESULTS
    LAST_RESULTS = res
    total = 0.0
    for r in res.results:
        total += float(np.sum(r["out"], dtype=np.float64))
    return np.array(total / n_pos, dtype=np.float32)


LAST_RESULTS = None


# revision 3
# speedup vs baseline: 1.1910x; 1.1910x over previous
"""Trainium2 Bass kernel for nn_ContrastiveLoss (NT-Xent style contrastive loss).

Strategy (8 NeuronCores, SPMD):
  - Host sorts samples by label (the scalar loss is permutation invariant),
    row-normalizes, and builds X^T [D=128, N=8192] in bf16.
  - Rows are sharded across 8 cores (1024 rows each, 8 blocks of 128).
  - Each core computes its [1024, 8192] similarity block against the full
    X^T (the "all-gathered" copy arrives as a per-core input), reduces
    exp-row-sums on-chip, and evaluates the positive-pair terms only on a
    narrow label-band window (sorted labels make positives contiguous).
  - Per-row partial losses return to the host, which sums them and divides
    by the exact positive-pair count (from the label histogram).

Math: with e_ij = exp(sim_ij/T), S_i = sum_j e_ij (incl diag),
P_i = sum_{j in label-range(i)} e_ij (incl diag), unsim_i = S_i - P_i,
u_i = log(unsim_i), the reference loss row-sum equals
  npos_i*u_i + sum_{range} softplus(sim_ij/T - u_i) - softplus(1/T - u_i)
             - (sum_{range} sim_ij/T - 1/T)
where npos_i = (label count of i) - 1. The diagonal contributions cancel
exactly in unsim and are removed via the constant sim_ii = 1 (rows are
normalized; the fp difference is ~1e-9 relative on the final scalar).

Perf structure (v2):
  - One activation-table set (natural_log_exp_and_others) serves both Exp
    and Ln, so the ACT engine never reloads tables mid-stream regardless
    of how the tile scheduler interleaves the two.
  - Fully per-block pipeline: each block's band/log tail overlaps the next
    block's dense Exp stream, keeping ACT (the bottleneck engine) busy.
  - Dense Exp runs in-place on PSUM with accum_out row-sums; the exp
    values are never copied to SBUF.
  - Tiny Ln ops (u = ln(unsim), spd = softplus diag) are batched into one
    [128, 16] Ln at the end.
  - Masked band reductions use scalar_tensor_tensor accum_out (one DVE op
    instead of multiply + reduce).
"""

import numpy as np

T = 0.2
INV_T = 1.0 / T  # 5.0
EPS = 1e-5
N, D, NCLASS = 8192, 128, 128
NCORES = 8
ROWS_PER_CORE = N // NCORES          # 1024
BLOCKS = ROWS_PER_CORE // 128        # 8 blocks of 128 rows per core
CHUNK = 2048                         # ACT chunk (4 PSUM banks)
NCHUNKS = N // CHUNK                 # 4 per block
MM = 512                             # matmul free-dim per PSUM bank

_CACHE = {}


def _force_single_act_table_set():
    """Make every activation resolve to natural_log_exp_and_others (which
    contains both Exp and Ln) so the ACT engine loads its spline tables
    exactly once. get_activation_tables is functools.cache'd and returns
    the same dict object to all consumers (bacc's insert_act_table_loads,
    the scheduler cost model), so in-place mutation is seen everywhere.
    Set ids stay valid: keys/order are untouched, other sets just become
    empty and thus never get selected."""
    try:
        from concourse.hw_specs import get_activation_tables
        for arch in ("gen3",):  # Bacc("TRN2") → module arch "gen3"
            tabs = get_activation_tables(arch)
            if "natural_log_exp_and_others" not in tabs:
                continue
            for name in list(tabs.keys()):
                if name != "natural_log_exp_and_others":
                    tabs[name] = set()
    except Exception:
        pass  # fall back to default (correct, just slower) table handling


def _build_nc(W, debug=False):
    """Build the SPMD Bass/Tile program. W = band window width (mult of 256)."""
    import concourse.bass as bass
    import concourse.bacc as bacc
    import concourse.mybir as mybir
    import concourse.tile as tile

    _force_single_act_table_set()

    dt = mybir.dt
    AF = mybir.ActivationFunctionType
    ALU = mybir.AluOpType
    X = mybir.AxisListType.X

    nc = bacc.Bacc("TRN2", target_bir_lowering=False, debug=debug)

    xt_d = nc.dram_tensor("xt", [128, N], dt.bfloat16, kind="ExternalInput")
    xtown_d = nc.dram_tensor("xtown", [128, ROWS_PER_CORE], dt.bfloat16,
                             kind="ExternalInput")
    xtband_d = nc.dram_tensor("xtband", [128, BLOCKS * W], dt.bfloat16,
                              kind="ExternalInput")
    gsr_d = nc.dram_tensor("gsr", [128, BLOCKS], dt.float32, kind="ExternalInput")
    ger_d = nc.dram_tensor("ger", [128, BLOCKS], dt.float32, kind="ExternalInput")
    npos_d = nc.dram_tensor("npos", [128, BLOCKS], dt.float32, kind="ExternalInput")
    out_d = nc.dram_tensor("out", [128, BLOCKS], dt.float32, kind="ExternalOutput")

    E5 = float(np.exp(INV_T))

    with tile.TileContext(nc) as tc:
        with (
            tc.tile_pool(name="const", bufs=1) as const,
            tc.tile_pool(name="band", bufs=1) as band,
            tc.tile_pool(name="scr", bufs=3) as scr_pool,
            tc.tile_pool(name="small", bufs=1) as small,
            tc.tile_pool(name="psum", bufs=2, space="PSUM") as psum,
        ):
            # ---- persistent loads (first-needed first) ----
            xtown = const.tile([128, ROWS_PER_CORE], dt.bfloat16)
            nc.sync.dma_start(xtown[:], xtown_d[:])
            xt = const.tile([128, N], dt.bfloat16)
            for k in range(N // CHUNK):
                nc.sync.dma_start(xt[:, k * CHUNK:(k + 1) * CHUNK],
                                  xt_d[:, k * CHUNK:(k + 1) * CHUNK])
            xtband = const.tile([128, BLOCKS * W], dt.bfloat16)
            nc.sync.dma_start(xtband[:], xtband_d[:])
            gsr = const.tile([128, BLOCKS], dt.float32)
            nc.sync.dma_start(gsr[:], gsr_d[:])
            ger = const.tile([128, BLOCKS], dt.float32)
            nc.sync.dma_start(ger[:], ger_d[:])
            npos = const.tile([128, BLOCKS], dt.float32)
            nc.sync.dma_start(npos[:], npos_d[:])

            iota_i = const.tile([128, W], dt.int32)
            nc.gpsimd.iota(iota_i[:], pattern=[[1, W]], base=0, channel_multiplier=0)
            iota_f = const.tile([128, W], dt.float32)
            nc.vector.tensor_copy(iota_f[:], iota_i[:])

            # per-block persistent tiles
            mask = [band.tile([128, W], dt.float32, name=f"mk{b}") for b in range(BLOCKS)]
            e_band = [band.tile([128, W], dt.float32, name=f"eb{b}") for b in range(BLOCKS)]

            # small per-row accumulators, one column per block
            S8 = small.tile([128, BLOCKS], dt.float32, name="S8")
            P8 = small.tile([128, BLOCKS], dt.float32, name="P8")
            A8 = small.tile([128, BLOCKS], dt.float32, name="A8")
            B8 = small.tile([128, BLOCKS], dt.float32, name="B8")
            runsim8 = small.tile([128, BLOCKS], dt.float32, name="rs8")
            # v16 cols 0..7 = unsim per block; cols 8..15 = 1 + E5*runsim
            v16 = small.tile([128, 2 * BLOCKS], dt.float32, name="v16")
            ln16 = small.tile([128, 2 * BLOCKS], dt.float32, name="ln16")
            acc = small.tile([128, BLOCKS], dt.float32, name="acc")

            # ---- masks upfront (DVE only; runs during DMA/matmul fill) ----
            for b in range(BLOCKS):
                m1 = scr_pool.tile([128, W], dt.float32, tag="m1")
                nc.vector.tensor_scalar(m1[:], iota_f[:], gsr[:, b:b + 1], None,
                                        op0=ALU.is_ge)
                nc.vector.scalar_tensor_tensor(mask[b][:], iota_f[:],
                                               ger[:, b:b + 1], m1[:],
                                               op0=ALU.is_lt, op1=ALU.mult)

            # ---- per-block pipeline ----
            for b in range(BLOCKS):
                lhsT = xtown[:, b * 128:(b + 1) * 128]
                sparts = scr_pool.tile([128, NCHUNKS], dt.float32, tag="sparts")
                for kc in range(NCHUNKS):
                    ps = psum.tile([128, CHUNK], dt.float32, tag="ps")
                    for j in range(CHUNK // MM):
                        c0 = kc * CHUNK + j * MM
                        nc.tensor.matmul(ps[:, j * MM:(j + 1) * MM], lhsT,
                                         xt[:, c0:c0 + MM], start=True, stop=True)
                    # Exp in place on PSUM; only the row-sum survives.
                    nc.scalar.activation(ps[:], ps[:], AF.Exp, bias=0.0,
                                         scale=INV_T,
                                         accum_out=sparts[:, kc:kc + 1])
                # band: sims for the W-wide positive window
                psb = psum.tile([128, W], dt.float32, tag="ps")
                nc.tensor.matmul(psb[:, :W], lhsT,
                                 xtband[:, b * W:(b + 1) * W],
                                 start=True, stop=True)
                nc.scalar.activation(e_band[b][:], psb[:, :W], AF.Exp, bias=0.0,
                                     scale=INV_T)
                # B = sum_range sim (read raw sims straight from PSUM)
                scrB = scr_pool.tile([128, W], dt.float32, tag="scrB")
                nc.vector.scalar_tensor_tensor(scrB[:], psb[:, :W], 0.0,
                                               mask[b][:], op0=ALU.bypass,
                                               op1=ALU.mult,
                                               accum_out=B8[:, b:b + 1])
                # S, P, unsim, runsim, 1 + E5*runsim
                nc.vector.reduce_sum(S8[:, b:b + 1], sparts[:], axis=X)
                scrP = scr_pool.tile([128, W], dt.float32, tag="scrP")
                nc.vector.scalar_tensor_tensor(scrP[:], e_band[b][:], 0.0,
                                               mask[b][:], op0=ALU.bypass,
                                               op1=ALU.mult,
                                               accum_out=P8[:, b:b + 1])
                nc.vector.tensor_sub(v16[:, b:b + 1], S8[:, b:b + 1],
                                     P8[:, b:b + 1])
                nc.vector.reciprocal(runsim8[:, b:b + 1], v16[:, b:b + 1])
                nc.vector.tensor_scalar(v16[:, BLOCKS + b:BLOCKS + b + 1],
                                        runsim8[:, b:b + 1], E5, 1.0,
                                        op0=ALU.mult, op1=ALU.add)
                # softplus stream: sp = Ln(1 + e_band*runsim), A = masked sum
                nc.vector.tensor_scalar(e_band[b][:], e_band[b][:],
                                        runsim8[:, b:b + 1], None, op0=ALU.mult)
                sp = scr_pool.tile([128, W], dt.float32, tag="sp")
                nc.scalar.activation(sp[:], e_band[b][:], AF.Ln, bias=1.0)
                scrA = scr_pool.tile([128, W], dt.float32, tag="scrA")
                nc.vector.scalar_tensor_tensor(scrA[:], sp[:], 0.0, mask[b][:],
                                               op0=ALU.bypass, op1=ALU.mult,
                                               accum_out=A8[:, b:b + 1])

            # ---- batched logs + final combine ----
            nc.scalar.activation(ln16[:], v16[:], AF.Ln)  # u8 | ln(1+E5*r)
            u8 = ln16[:, 0:BLOCKS]
            spd8 = ln16[:, BLOCKS:2 * BLOCKS]
            ta = small.tile([128, BLOCKS], dt.float32, name="ta")
            nc.vector.tensor_mul(ta[:], npos[:], u8)
            tb = small.tile([128, BLOCKS], dt.float32, name="tb")
            nc.vector.tensor_add(tb[:], ta[:], A8[:])
            r2 = small.tile([128, BLOCKS], dt.float32, name="r2")
            nc.vector.tensor_scalar(r2[:], B8[:], INV_T, -INV_T,
                                    op0=ALU.mult, op1=ALU.add)
            r3 = small.tile([128, BLOCKS], dt.float32, name="r3")
            nc.vector.tensor_add(r3[:], r2[:], spd8)
            nc.vector.tensor_sub(acc[:], tb[:], r3[:])

            nc.sync.dma_start(out_d[:], acc[:])

    nc.compile()
    return nc


def _prep(input, label):
    """Host-side shard prep: sort by label, normalize, build per-core inputs."""
    import ml_dtypes

    x = np.asarray(input, dtype=np.float32).reshape(N, D)
    lab = np.asarray(label).astype(np.int64).reshape(N)

    order = np.argsort(lab, kind="stable")
    xs, ls = x[order], lab[order]
    counts = np.bincount(ls, minlength=NCLASS)
    n_pos = int((counts.astype(np.int64) ** 2).sum()) - N
    ends = np.cumsum(counts)
    starts = ends - counts
    row_gs = starts[ls]          # [N] group start col per (sorted) row
    row_ge = ends[ls]            # [N] group end col per row

    norms = np.sqrt((xs * xs).sum(1, dtype=np.float32)).astype(np.float32)
    # reference divides by max(n_i*n_j, EPS); for this data the max never
    # binds (norms ~ 11), so plain normalization is exact.
    assert float(norms.min()) ** 2 > EPS * 1.0001
    xn = (xs / norms[:, None]).astype(np.float32)
    xt = np.ascontiguousarray(xn.T).astype(ml_dtypes.bfloat16)  # [128, N]

    # band windows per global block
    nblk = N // 128
    lo = row_gs[np.arange(nblk) * 128]
    hi = row_ge[np.arange(nblk) * 128 + 127]
    maxband = int((hi - lo).max())
    W = max(256, ((maxband + 255) // 256) * 256)
    wstart = np.minimum(lo, N - W)

    in_maps = []
    for c in range(NCORES):
        r0 = c * ROWS_PER_CORE
        xtband = np.empty((128, BLOCKS * W), dtype=ml_dtypes.bfloat16)
        gsr = np.empty((128, BLOCKS), np.float32)
        ger = np.empty((128, BLOCKS), np.float32)
        npos = np.empty((128, BLOCKS), np.float32)
        for b in range(BLOCKS):
            g = c * BLOCKS + b
            ws = int(wstart[g])
            xtband[:, b * W:(b + 1) * W] = xt[:, ws:ws + W]
            rows = slice(r0 + b * 128, r0 + (b + 1) * 128)
            gsr[:, b] = (row_gs[rows] - ws).astype(np.float32)
            ger[:, b] = (row_ge[rows] - ws).astype(np.float32)
            npos[:, b] = (row_ge[rows] - row_gs[rows] - 1).astype(np.float32)
        in_maps.append({
            "xt": xt,
            "xtown": np.ascontiguousarray(
                xt[:, r0:r0 + ROWS_PER_CORE]),
            "xtband": xtband,
            "gsr": gsr,
            "ger": ger,
            "npos": npos,
        })
    return in_maps, n_pos, W


def kernel(input, label):
    from concourse.bass_utils import run_bass_kernel_spmd

    in_maps, n_pos, W = _prep(input, label)
    if W not in _CACHE:
        _CACHE[W] = _build_nc(W)
    nc = _CACHE[W]

    res = None
    for attempt in range(4):
        try:
            res = run_bass_kernel_spmd(nc, in_maps, core_ids=list(range(NCORES)))
            break
        except Exception:
            if attempt == 3:
                raise
            import time
            time.sleep(45)  # device may need a moment to recover
    global LAST_RESULTS
    LAST_RESULTS = res
    total = 0.0
    for r in res.results:
        total += float(np.sum(r["out"], dtype=np.float64))
    return np.array(total / n_pos, dtype=np.float32)


LAST_RESULTS = None


# revision 5
# speedup vs baseline: 1.2033x; 1.0104x over previous
"""Trainium2 Bass kernel for nn_ContrastiveLoss (NT-Xent style contrastive loss).

Strategy (8 NeuronCores, SPMD):
  - Host sorts samples by label (the scalar loss is permutation invariant),
    row-normalizes, and builds X^T [D=128, N=8192] in bf16.
  - Rows are sharded across 8 cores (1024 rows each, 8 blocks of 128).
  - Each core computes its [1024, 8192] similarity block against the full
    X^T (the "all-gathered" copy arrives as a per-core input), reduces
    exp-row-sums on-chip, and evaluates the positive-pair terms only on a
    narrow label-band window (sorted labels make positives contiguous).
  - Per-row partial losses return to the host, which sums them and divides
    by the exact positive-pair count (from the label histogram).

Math: with e_ij = exp(sim_ij/T), S_i = sum_j e_ij (incl diag),
P_i = sum_{j in label-range(i)} e_ij (incl diag), unsim_i = S_i - P_i,
u_i = log(unsim_i), the reference loss row-sum equals
  npos_i*u_i + sum_{range} softplus(sim_ij/T - u_i) - softplus(1/T - u_i)
             - (sum_{range} sim_ij/T - 1/T)
where npos_i = (label count of i) - 1. The diagonal contributions cancel
exactly in unsim and are removed via the constant sim_ii = 1 (rows are
normalized; the fp difference is ~1e-9 relative on the final scalar).

Perf structure (v2):
  - One activation-table set (natural_log_exp_and_others) serves both Exp
    and Ln, so the ACT engine never reloads tables mid-stream regardless
    of how the tile scheduler interleaves the two.
  - Fully per-block pipeline: each block's band/log tail overlaps the next
    block's dense Exp stream, keeping ACT (the bottleneck engine) busy.
  - Dense Exp runs in-place on PSUM with accum_out row-sums; the exp
    values are never copied to SBUF.
  - Tiny Ln ops (u = ln(unsim), spd = softplus diag) are batched into one
    [128, 16] Ln at the end.
  - Masked band reductions use scalar_tensor_tensor accum_out (one DVE op
    instead of multiply + reduce).
"""

import numpy as np

T = 0.2
INV_T = 1.0 / T  # 5.0
EPS = 1e-5
N, D, NCLASS = 8192, 128, 128
NCORES = 8
ROWS_PER_CORE = N // NCORES          # 1024
BLOCKS = ROWS_PER_CORE // 128        # 8 blocks of 128 rows per core
CHUNK = 2048                         # ACT chunk (4 PSUM banks)
NCHUNKS = N // CHUNK                 # 4 per block
MM = 512                             # matmul free-dim per PSUM bank

_CACHE = {}


def _force_single_act_table_set():
    """Make every activation resolve to natural_log_exp_and_others (which
    contains both Exp and Ln) so the ACT engine loads its spline tables
    exactly once. get_activation_tables is functools.cache'd and returns
    the same dict object to all consumers (bacc's insert_act_table_loads,
    the scheduler cost model), so in-place mutation is seen everywhere.
    Set ids stay valid: keys/order are untouched, other sets just become
    empty and thus never get selected."""
    try:
        from concourse.hw_specs import get_activation_tables
        for arch in ("gen3",):  # Bacc("TRN2") → module arch "gen3"
            tabs = get_activation_tables(arch)
            if "natural_log_exp_and_others" not in tabs:
                continue
            for name in list(tabs.keys()):
                if name != "natural_log_exp_and_others":
                    tabs[name] = set()
    except Exception:
        pass  # fall back to default (correct, just slower) table handling


def _build_nc(W, debug=False):
    """Build the SPMD Bass/Tile program. W = band window width (mult of 256)."""
    import concourse.bass as bass
    import concourse.bacc as bacc
    import concourse.mybir as mybir
    import concourse.tile as tile

    _force_single_act_table_set()

    dt = mybir.dt
    AF = mybir.ActivationFunctionType
    ALU = mybir.AluOpType
    X = mybir.AxisListType.X

    nc = bacc.Bacc("TRN2", target_bir_lowering=False, debug=debug)

    xt_d = nc.dram_tensor("xt", [128, N], dt.bfloat16, kind="ExternalInput")
    xtown_d = nc.dram_tensor("xtown", [128, ROWS_PER_CORE], dt.bfloat16,
                             kind="ExternalInput")
    xtband_d = nc.dram_tensor("xtband", [128, BLOCKS * W], dt.bfloat16,
                              kind="ExternalInput")
    gsr_d = nc.dram_tensor("gsr", [128, BLOCKS], dt.float32, kind="ExternalInput")
    ger_d = nc.dram_tensor("ger", [128, BLOCKS], dt.float32, kind="ExternalInput")
    npos_d = nc.dram_tensor("npos", [128, BLOCKS], dt.float32, kind="ExternalInput")
    out_d = nc.dram_tensor("out", [128, BLOCKS], dt.float32, kind="ExternalOutput")

    E5 = float(np.exp(INV_T))

    with tile.TileContext(nc) as tc:
        with (
            tc.tile_pool(name="const", bufs=1) as const,
            tc.tile_pool(name="band", bufs=1) as band,
            tc.tile_pool(name="scr", bufs=3) as scr_pool,
            tc.tile_pool(name="small", bufs=1) as small,
            tc.tile_pool(name="psum", bufs=2, space="PSUM") as psum,
        ):
            # ---- persistent loads (first-needed first; small first pieces so
            # the first matmul/exp starts as early as possible) ----
            xtown = const.tile([128, ROWS_PER_CORE], dt.bfloat16)
            nc.sync.dma_start(xtown[:, 0:128], xtown_d[:, 0:128])
            xt = const.tile([128, N], dt.bfloat16)
            xt_cuts = [0, 512, 1024, 2048, 4096, 6144, N]
            for a, bnd in zip(xt_cuts[:-1], xt_cuts[1:]):
                nc.sync.dma_start(xt[:, a:bnd], xt_d[:, a:bnd])
            nc.sync.dma_start(xtown[:, 128:], xtown_d[:, 128:])
            xtband = const.tile([128, BLOCKS * W], dt.bfloat16)
            nc.sync.dma_start(xtband[:], xtband_d[:])
            gsr = const.tile([128, BLOCKS], dt.float32)
            nc.sync.dma_start(gsr[:], gsr_d[:])
            ger = const.tile([128, BLOCKS], dt.float32)
            nc.sync.dma_start(ger[:], ger_d[:])
            npos = const.tile([128, BLOCKS], dt.float32)
            nc.sync.dma_start(npos[:], npos_d[:])

            iota_i = const.tile([128, W], dt.int32)
            nc.gpsimd.iota(iota_i[:], pattern=[[1, W]], base=0, channel_multiplier=0)
            iota_f = const.tile([128, W], dt.float32)
            nc.vector.tensor_copy(iota_f[:], iota_i[:])

            # per-block persistent tiles
            mask = [band.tile([128, W], dt.float32, name=f"mk{b}") for b in range(BLOCKS)]
            e_band = [band.tile([128, W], dt.float32, name=f"eb{b}") for b in range(BLOCKS)]

            # small per-row accumulators, one column per block
            S8 = small.tile([128, BLOCKS], dt.float32, name="S8")
            P8 = small.tile([128, BLOCKS], dt.float32, name="P8")
            A8 = small.tile([128, BLOCKS], dt.float32, name="A8")
            B8 = small.tile([128, BLOCKS], dt.float32, name="B8")
            runsim8 = small.tile([128, BLOCKS], dt.float32, name="rs8")
            # v16 cols 0..7 = unsim per block; cols 8..15 = 1 + E5*runsim
            v16 = small.tile([128, 2 * BLOCKS], dt.float32, name="v16")
            ln16 = small.tile([128, 2 * BLOCKS], dt.float32, name="ln16")
            acc = small.tile([128, BLOCKS], dt.float32, name="acc")

            # ---- masks upfront (DVE only; runs during DMA/matmul fill) ----
            for b in range(BLOCKS):
                m1 = scr_pool.tile([128, W], dt.float32, tag="m1")
                nc.vector.tensor_scalar(m1[:], iota_f[:], gsr[:, b:b + 1], None,
                                        op0=ALU.is_ge)
                nc.vector.scalar_tensor_tensor(mask[b][:], iota_f[:],
                                               ger[:, b:b + 1], m1[:],
                                               op0=ALU.is_lt, op1=ALU.mult)

            # ---- per-block pipeline ----
            # The softplus Ln of block b is emitted during block b+1 (after
            # that block's band Exp) so its DVE-produced input is long ready
            # by the time the ACT engine reaches it: the ACT stream stays
            # gapless. Block 0's first chunks are split small so the first
            # Exp starts as soon as the first DMA pieces land.
            def softplus_tail(b):
                # sp = Ln(1 + e_band*runsim)  (e_band holds e*runsim by now)
                sp = scr_pool.tile([128, W], dt.float32, tag="sp")
                nc.scalar.activation(sp[:], e_band[b][:], AF.Ln, bias=1.0)
                scrA = scr_pool.tile([128, W], dt.float32, tag="scrA")
                nc.vector.scalar_tensor_tensor(scrA[:], sp[:], 0.0, mask[b][:],
                                               op0=ALU.bypass, op1=ALU.mult,
                                               accum_out=A8[:, b:b + 1])

            for b in range(BLOCKS):
                lhsT = xtown[:, b * 128:(b + 1) * 128]
                cuts = [0, 512, 1024, 2048, 4096, 6144, N] if b == 0 else \
                    list(range(0, N + 1, CHUNK))
                nparts = len(cuts) - 1
                sparts = scr_pool.tile([128, nparts], dt.float32,
                                       tag=f"sparts{nparts}")
                for kc in range(nparts):
                    lo, hi = cuts[kc], cuts[kc + 1]
                    ps = psum.tile([128, hi - lo], dt.float32, tag="ps")
                    for j in range(0, hi - lo, MM):
                        nc.tensor.matmul(ps[:, j:j + min(MM, hi - lo - j)],
                                         lhsT,
                                         xt[:, lo + j:min(lo + j + MM, hi)],
                                         start=True, stop=True)
                    # Exp in place on PSUM; only the row-sum survives.
                    nc.scalar.activation(ps[:], ps[:], AF.Exp, bias=0.0,
                                         scale=INV_T,
                                         accum_out=sparts[:, kc:kc + 1])
                # band: sims for the W-wide positive window
                psb = psum.tile([128, W], dt.float32, tag="ps")
                nc.tensor.matmul(psb[:, :W], lhsT,
                                 xtband[:, b * W:(b + 1) * W],
                                 start=True, stop=True)
                nc.scalar.activation(e_band[b][:], psb[:, :W], AF.Exp, bias=0.0,
                                     scale=INV_T)
                if b > 0:
                    softplus_tail(b - 1)
                # B = sum_range sim (read raw sims straight from PSUM)
                scrB = scr_pool.tile([128, W], dt.float32, tag="scrB")
                nc.vector.scalar_tensor_tensor(scrB[:], psb[:, :W], 0.0,
                                               mask[b][:], op0=ALU.bypass,
                                               op1=ALU.mult,
                                               accum_out=B8[:, b:b + 1])
                # S, P, unsim, runsim, 1 + E5*runsim
                nc.vector.reduce_sum(S8[:, b:b + 1], sparts[:], axis=X)
                scrP = scr_pool.tile([128, W], dt.float32, tag="scrP")
                nc.vector.scalar_tensor_tensor(scrP[:], e_band[b][:], 0.0,
                                               mask[b][:], op0=ALU.bypass,
                                               op1=ALU.mult,
                                               accum_out=P8[:, b:b + 1])
                nc.vector.tensor_sub(v16[:, b:b + 1], S8[:, b:b + 1],
                                     P8[:, b:b + 1])
                nc.vector.reciprocal(runsim8[:, b:b + 1], v16[:, b:b + 1])
                nc.vector.tensor_scalar(v16[:, BLOCKS + b:BLOCKS + b + 1],
                                        runsim8[:, b:b + 1], E5, 1.0,
                                        op0=ALU.mult, op1=ALU.add)
                nc.vector.tensor_scalar(e_band[b][:], e_band[b][:],
                                        runsim8[:, b:b + 1], None, op0=ALU.mult)
            softplus_tail(BLOCKS - 1)

            # ---- batched logs + final combine ----
            nc.scalar.activation(ln16[:], v16[:], AF.Ln)  # u8 | ln(1+E5*r)
            u8 = ln16[:, 0:BLOCKS]
            spd8 = ln16[:, BLOCKS:2 * BLOCKS]
            ta = small.tile([128, BLOCKS], dt.float32, name="ta")
            nc.vector.tensor_mul(ta[:], npos[:], u8)
            tb = small.tile([128, BLOCKS], dt.float32, name="tb")
            nc.vector.tensor_add(tb[:], ta[:], A8[:])
            r2 = small.tile([128, BLOCKS], dt.float32, name="r2")
            nc.vector.tensor_scalar(r2[:], B8[:], INV_T, -INV_T,
                                    op0=ALU.mult, op1=ALU.add)
            r3 = small.tile([128, BLOCKS], dt.float32, name="r3")
            nc.vector.tensor_add(r3[:], r2[:], spd8)
            nc.vector.tensor_sub(acc[:], tb[:], r3[:])

            nc.sync.dma_start(out_d[:], acc[:])

    nc.compile()
    return nc


def _prep(input, label):
    """Host-side shard prep: sort by label, normalize, build per-core inputs."""
    import ml_dtypes

    x = np.asarray(input, dtype=np.float32).reshape(N, D)
    lab = np.asarray(label).astype(np.int64).reshape(N)

    order = np.argsort(lab, kind="stable")
    xs, ls = x[order], lab[order]
    counts = np.bincount(ls, minlength=NCLASS)
    n_pos = int((counts.astype(np.int64) ** 2).sum()) - N
    ends = np.cumsum(counts)
    starts = ends - counts
    row_gs = starts[ls]          # [N] group start col per (sorted) row
    row_ge = ends[ls]            # [N] group end col per row

    norms = np.sqrt((xs * xs).sum(1, dtype=np.float32)).astype(np.float32)
    # reference divides by max(n_i*n_j, EPS); for this data the max never
    # binds (norms ~ 11), so plain normalization is exact.
    assert float(norms.min()) ** 2 > EPS * 1.0001
    xn = (xs / norms[:, None]).astype(np.float32)
    xt = np.ascontiguousarray(xn.T).astype(ml_dtypes.bfloat16)  # [128, N]

    # band windows per global block
    nblk = N // 128
    lo = row_gs[np.arange(nblk) * 128]
    hi = row_ge[np.arange(nblk) * 128 + 127]
    maxband = int((hi - lo).max())
    W = max(256, ((maxband + 255) // 256) * 256)
    wstart = np.minimum(lo, N - W)

    in_maps = []
    for c in range(NCORES):
        r0 = c * ROWS_PER_CORE
        xtband = np.empty((128, BLOCKS * W), dtype=ml_dtypes.bfloat16)
        gsr = np.empty((128, BLOCKS), np.float32)
        ger = np.empty((128, BLOCKS), np.float32)
        npos = np.empty((128, BLOCKS), np.float32)
        for b in range(BLOCKS):
            g = c * BLOCKS + b
            ws = int(wstart[g])
            xtband[:, b * W:(b + 1) * W] = xt[:, ws:ws + W]
            rows = slice(r0 + b * 128, r0 + (b + 1) * 128)
            gsr[:, b] = (row_gs[rows] - ws).astype(np.float32)
            ger[:, b] = (row_ge[rows] - ws).astype(np.float32)
            npos[:, b] = (row_ge[rows] - row_gs[rows] - 1).astype(np.float32)
        in_maps.append({
            "xt": xt,
            "xtown": np.ascontiguousarray(
                xt[:, r0:r0 + ROWS_PER_CORE]),
            "xtband": xtband,
            "gsr": gsr,
            "ger": ger,
            "npos": npos,
        })
    return in_maps, n_pos, W


def kernel(input, label):
    from concourse.bass_utils import run_bass_kernel_spmd

    in_maps, n_pos, W = _prep(input, label)
    if W not in _CACHE:
        _CACHE[W] = _build_nc(W)
    nc = _CACHE[W]

    res = None
    for attempt in range(4):
        try:
            res = run_bass_kernel_spmd(nc, in_maps, core_ids=list(range(NCORES)))
            break
        except Exception:
            if attempt == 3:
                raise
            import time
            time.sleep(45)  # device may need a moment to recover
    global LAST_RESULTS
    LAST_RESULTS = res
    total = 0.0
    for r in res.results:
        total += float(np.sum(r["out"], dtype=np.float64))
    return np.array(total / n_pos, dtype=np.float32)


LAST_RESULTS = None


# revision 7
# speedup vs baseline: 1.2075x; 1.0034x over previous
"""Trainium2 Bass kernel for nn_ContrastiveLoss (NT-Xent style contrastive loss).

Strategy (8 NeuronCores, SPMD):
  - Host sorts samples by label (the scalar loss is permutation invariant),
    row-normalizes, and builds X^T [D=128, N=8192] in bf16.
  - Rows are sharded across 8 cores (1024 rows each, 8 blocks of 128).
  - Each core computes its [1024, 8192] similarity block against the full
    X^T (the "all-gathered" copy arrives as a per-core input), reduces
    exp-row-sums on-chip, and evaluates the positive-pair terms only on a
    narrow label-band window (sorted labels make positives contiguous).
  - Per-row partial losses return to the host, which sums them and divides
    by the exact positive-pair count (from the label histogram).

Math: with e_ij = exp(sim_ij/T), S_i = sum_j e_ij (incl diag),
P_i = sum_{j in label-range(i)} e_ij (incl diag), unsim_i = S_i - P_i,
u_i = log(unsim_i), the reference loss row-sum equals
  npos_i*u_i + sum_{range} softplus(sim_ij/T - u_i) - softplus(1/T - u_i)
             - (sum_{range} sim_ij/T - 1/T)
where npos_i = (label count of i) - 1. The diagonal contributions cancel
exactly in unsim and are removed via the constant sim_ii = 1 (rows are
normalized; the fp difference is ~1e-9 relative on the final scalar).

Perf structure (v2):
  - One activation-table set (natural_log_exp_and_others) serves both Exp
    and Ln, so the ACT engine never reloads tables mid-stream regardless
    of how the tile scheduler interleaves the two.
  - Fully per-block pipeline: each block's band/log tail overlaps the next
    block's dense Exp stream, keeping ACT (the bottleneck engine) busy.
  - Dense Exp runs in-place on PSUM with accum_out row-sums; the exp
    values are never copied to SBUF.
  - Tiny Ln ops (u = ln(unsim), spd = softplus diag) are batched into one
    [128, 16] Ln at the end.
  - Masked band reductions use scalar_tensor_tensor accum_out (one DVE op
    instead of multiply + reduce).
"""

import numpy as np

T = 0.2
INV_T = 1.0 / T  # 5.0
EPS = 1e-5
N, D, NCLASS = 8192, 128, 128
NCORES = 8
ROWS_PER_CORE = N // NCORES          # 1024
BLOCKS = ROWS_PER_CORE // 128        # 8 blocks of 128 rows per core
CHUNK = 2048                         # ACT chunk (4 PSUM banks)
NCHUNKS = N // CHUNK                 # 4 per block
MM = 512                             # matmul free-dim per PSUM bank

_CACHE = {}


def _force_single_act_table_set():
    """Make every activation resolve to natural_log_exp_and_others (which
    contains both Exp and Ln) so the ACT engine loads its spline tables
    exactly once. get_activation_tables is functools.cache'd and returns
    the same dict object to all consumers (bacc's insert_act_table_loads,
    the scheduler cost model), so in-place mutation is seen everywhere.
    Set ids stay valid: keys/order are untouched, other sets just become
    empty and thus never get selected."""
    try:
        from concourse.hw_specs import get_activation_tables
        for arch in ("gen3",):  # Bacc("TRN2") → module arch "gen3"
            tabs = get_activation_tables(arch)
            if "natural_log_exp_and_others" not in tabs:
                continue
            for name in list(tabs.keys()):
                if name != "natural_log_exp_and_others":
                    tabs[name] = set()
    except Exception:
        pass  # fall back to default (correct, just slower) table handling


def _build_nc(W, debug=False):
    """Build the SPMD Bass/Tile program. W = band window width (mult of 256)."""
    import concourse.bass as bass
    import concourse.bacc as bacc
    import concourse.mybir as mybir
    import concourse.tile as tile

    _force_single_act_table_set()

    dt = mybir.dt
    AF = mybir.ActivationFunctionType
    ALU = mybir.AluOpType
    X = mybir.AxisListType.X

    nc = bacc.Bacc("TRN2", target_bir_lowering=False, debug=debug)

    xt_d = nc.dram_tensor("xt", [128, N], dt.bfloat16, kind="ExternalInput")
    xtown_d = nc.dram_tensor("xtown", [128, ROWS_PER_CORE], dt.bfloat16,
                             kind="ExternalInput")
    xtband_d = nc.dram_tensor("xtband", [128, BLOCKS * W], dt.bfloat16,
                              kind="ExternalInput")
    gsr_d = nc.dram_tensor("gsr", [128, BLOCKS], dt.float32, kind="ExternalInput")
    ger_d = nc.dram_tensor("ger", [128, BLOCKS], dt.float32, kind="ExternalInput")
    npos_d = nc.dram_tensor("npos", [128, BLOCKS], dt.float32, kind="ExternalInput")
    out_d = nc.dram_tensor("out", [128, BLOCKS], dt.float32, kind="ExternalOutput")

    E5 = float(np.exp(INV_T))

    with tile.TileContext(nc) as tc:
        with (
            tc.tile_pool(name="const", bufs=1) as const,
            tc.tile_pool(name="band", bufs=1) as band,
            tc.tile_pool(name="scr", bufs=3) as scr_pool,
            tc.tile_pool(name="small", bufs=1) as small,
            tc.tile_pool(name="psum", bufs=2, space="PSUM") as psum,
        ):
            # ---- persistent loads (first-needed first; small first pieces so
            # the first matmul/exp starts as early as possible) ----
            xtown = const.tile([128, ROWS_PER_CORE], dt.bfloat16)
            nc.sync.dma_start(xtown[:, 0:128], xtown_d[:, 0:128])
            xt = const.tile([128, N], dt.bfloat16)
            xt_cuts = [0, 512, 2048, 4096, 6144, N]
            for a, bnd in zip(xt_cuts[:-1], xt_cuts[1:]):
                nc.sync.dma_start(xt[:, a:bnd], xt_d[:, a:bnd])
            nc.sync.dma_start(xtown[:, 128:], xtown_d[:, 128:])
            xtband = const.tile([128, BLOCKS * W], dt.bfloat16)
            nc.sync.dma_start(xtband[:], xtband_d[:])
            gsr = const.tile([128, BLOCKS], dt.float32)
            nc.sync.dma_start(gsr[:], gsr_d[:])
            ger = const.tile([128, BLOCKS], dt.float32)
            nc.sync.dma_start(ger[:], ger_d[:])
            npos = const.tile([128, BLOCKS], dt.float32)
            nc.sync.dma_start(npos[:], npos_d[:])

            iota_i = const.tile([128, W], dt.int32)
            nc.gpsimd.iota(iota_i[:], pattern=[[1, W]], base=0, channel_multiplier=0)
            iota_f = const.tile([128, W], dt.float32)
            nc.vector.tensor_copy(iota_f[:], iota_i[:])

            # per-block persistent tiles
            mask = [band.tile([128, W], dt.float32, name=f"mk{b}") for b in range(BLOCKS)]
            e_band = [band.tile([128, W], dt.float32, name=f"eb{b}") for b in range(BLOCKS)]

            # small per-row accumulators, one column per block
            S8 = small.tile([128, BLOCKS], dt.float32, name="S8")
            P8 = small.tile([128, BLOCKS], dt.float32, name="P8")
            A8 = small.tile([128, BLOCKS], dt.float32, name="A8")
            B8 = small.tile([128, BLOCKS], dt.float32, name="B8")
            runsim8 = small.tile([128, BLOCKS], dt.float32, name="rs8")
            # v16 cols 0..7 = unsim per block; cols 8..15 = 1 + E5*runsim
            v16 = small.tile([128, 2 * BLOCKS], dt.float32, name="v16")
            ln16 = small.tile([128, 2 * BLOCKS], dt.float32, name="ln16")
            acc = small.tile([128, BLOCKS], dt.float32, name="acc")

            # ---- masks upfront (DVE only; runs during DMA/matmul fill) ----
            for b in range(BLOCKS):
                m1 = scr_pool.tile([128, W], dt.float32, tag="m1")
                nc.vector.tensor_scalar(m1[:], iota_f[:], gsr[:, b:b + 1], None,
                                        op0=ALU.is_ge)
                nc.vector.scalar_tensor_tensor(mask[b][:], iota_f[:],
                                               ger[:, b:b + 1], m1[:],
                                               op0=ALU.is_lt, op1=ALU.mult)

            # ---- per-block pipeline ----
            # The softplus Ln of block b is emitted during block b+1 (after
            # that block's band Exp) so its DVE-produced input is long ready
            # by the time the ACT engine reaches it: the ACT stream stays
            # gapless. Block 0's first chunks are split small so the first
            # Exp starts as soon as the first DMA pieces land.
            def softplus_tail(b):
                # sp = Ln(1 + e_band*runsim)  (e_band holds e*runsim by now)
                sp = scr_pool.tile([128, W], dt.float32, tag="sp")
                nc.scalar.activation(sp[:], e_band[b][:], AF.Ln, bias=1.0)
                scrA = scr_pool.tile([128, W], dt.float32, tag="scrA")
                nc.vector.scalar_tensor_tensor(scrA[:], sp[:], 0.0, mask[b][:],
                                               op0=ALU.bypass, op1=ALU.mult,
                                               accum_out=A8[:, b:b + 1])

            for b in range(BLOCKS):
                lhsT = xtown[:, b * 128:(b + 1) * 128]
                cuts = [0, 512, 2048, 4096, 6144, N] if b == 0 else \
                    list(range(0, N + 1, CHUNK))
                nparts = len(cuts) - 1
                sparts = scr_pool.tile([128, nparts], dt.float32,
                                       tag=f"sparts{nparts}")
                # The band matmul+exp is interleaved before the LAST dense
                # chunk: the short band-Exp + delayed softplus-Ln occupy ACT
                # while PE refills the final dense chunk, so ACT never
                # starves on the 2-deep PSUM ring.
                psb = None
                for kc in range(nparts):
                    if kc == nparts - 1:
                        psb = psum.tile([128, W], dt.float32, tag="ps")
                        nc.tensor.matmul(psb[:, :W], lhsT,
                                         xtband[:, b * W:(b + 1) * W],
                                         start=True, stop=True)
                        nc.scalar.activation(e_band[b][:], psb[:, :W], AF.Exp,
                                             bias=0.0, scale=INV_T)
                        if b > 0:
                            softplus_tail(b - 1)
                    lo, hi = cuts[kc], cuts[kc + 1]
                    ps = psum.tile([128, hi - lo], dt.float32, tag="ps")
                    for j in range(0, hi - lo, MM):
                        nc.tensor.matmul(ps[:, j:j + min(MM, hi - lo - j)],
                                         lhsT,
                                         xt[:, lo + j:min(lo + j + MM, hi)],
                                         start=True, stop=True)
                    # Exp in place on PSUM; only the row-sum survives.
                    nc.scalar.activation(ps[:], ps[:], AF.Exp, bias=0.0,
                                         scale=INV_T,
                                         accum_out=sparts[:, kc:kc + 1])
                # B = sum_range sim (read raw sims straight from PSUM)
                scrB = scr_pool.tile([128, W], dt.float32, tag="scrB")
                nc.vector.scalar_tensor_tensor(scrB[:], psb[:, :W], 0.0,
                                               mask[b][:], op0=ALU.bypass,
                                               op1=ALU.mult,
                                               accum_out=B8[:, b:b + 1])
                # S, P, unsim, runsim, 1 + E5*runsim
                nc.vector.reduce_sum(S8[:, b:b + 1], sparts[:], axis=X)
                scrP = scr_pool.tile([128, W], dt.float32, tag="scrP")
                nc.vector.scalar_tensor_tensor(scrP[:], e_band[b][:], 0.0,
                                               mask[b][:], op0=ALU.bypass,
                                               op1=ALU.mult,
                                               accum_out=P8[:, b:b + 1])
                nc.vector.tensor_sub(v16[:, b:b + 1], S8[:, b:b + 1],
                                     P8[:, b:b + 1])
                nc.vector.reciprocal(runsim8[:, b:b + 1], v16[:, b:b + 1])
                nc.vector.tensor_scalar(v16[:, BLOCKS + b:BLOCKS + b + 1],
                                        runsim8[:, b:b + 1], E5, 1.0,
                                        op0=ALU.mult, op1=ALU.add)
                nc.vector.tensor_scalar(e_band[b][:], e_band[b][:],
                                        runsim8[:, b:b + 1], None, op0=ALU.mult)
            softplus_tail(BLOCKS - 1)

            # ---- batched logs + final combine ----
            nc.scalar.activation(ln16[:], v16[:], AF.Ln)  # u8 | ln(1+E5*r)
            u8 = ln16[:, 0:BLOCKS]
            spd8 = ln16[:, BLOCKS:2 * BLOCKS]
            ta = small.tile([128, BLOCKS], dt.float32, name="ta")
            nc.vector.tensor_mul(ta[:], npos[:], u8)
            tb = small.tile([128, BLOCKS], dt.float32, name="tb")
            nc.vector.tensor_add(tb[:], ta[:], A8[:])
            r2 = small.tile([128, BLOCKS], dt.float32, name="r2")
            nc.vector.tensor_scalar(r2[:], B8[:], INV_T, -INV_T,
                                    op0=ALU.mult, op1=ALU.add)
            r3 = small.tile([128, BLOCKS], dt.float32, name="r3")
            nc.vector.tensor_add(r3[:], r2[:], spd8)
            nc.vector.tensor_sub(acc[:], tb[:], r3[:])

            nc.sync.dma_start(out_d[:], acc[:])

    nc.compile()
    return nc


def _prep(input, label):
    """Host-side shard prep: sort by label, normalize, build per-core inputs."""
    import ml_dtypes

    x = np.asarray(input, dtype=np.float32).reshape(N, D)
    lab = np.asarray(label).astype(np.int64).reshape(N)

    order = np.argsort(lab, kind="stable")
    xs, ls = x[order], lab[order]
    counts = np.bincount(ls, minlength=NCLASS)
    n_pos = int((counts.astype(np.int64) ** 2).sum()) - N
    ends = np.cumsum(counts)
    starts = ends - counts
    row_gs = starts[ls]          # [N] group start col per (sorted) row
    row_ge = ends[ls]            # [N] group end col per row

    norms = np.sqrt((xs * xs).sum(1, dtype=np.float32)).astype(np.float32)
    # reference divides by max(n_i*n_j, EPS); for this data the max never
    # binds (norms ~ 11), so plain normalization is exact.
    assert float(norms.min()) ** 2 > EPS * 1.0001
    xn = (xs / norms[:, None]).astype(np.float32)
    xt = np.ascontiguousarray(xn.T).astype(ml_dtypes.bfloat16)  # [128, N]

    # band windows per global block
    nblk = N // 128
    lo = row_gs[np.arange(nblk) * 128]
    hi = row_ge[np.arange(nblk) * 128 + 127]
    maxband = int((hi - lo).max())
    W = max(256, ((maxband + 255) // 256) * 256)
    wstart = np.minimum(lo, N - W)

    in_maps = []
    for c in range(NCORES):
        r0 = c * ROWS_PER_CORE
        xtband = np.empty((128, BLOCKS * W), dtype=ml_dtypes.bfloat16)
        gsr = np.empty((128, BLOCKS), np.float32)
        ger = np.empty((128, BLOCKS), np.float32)
        npos = np.empty((128, BLOCKS), np.float32)
        for b in range(BLOCKS):
            g = c * BLOCKS + b
            ws = int(wstart[g])
            xtband[:, b * W:(b + 1) * W] = xt[:, ws:ws + W]
            rows = slice(r0 + b * 128, r0 + (b + 1) * 128)
            gsr[:, b] = (row_gs[rows] - ws).astype(np.float32)
            ger[:, b] = (row_ge[rows] - ws).astype(np.float32)
            npos[:, b] = (row_ge[rows] - row_gs[rows] - 1).astype(np.float32)
        in_maps.append({
            "xt": xt,
            "xtown": np.ascontiguousarray(
                xt[:, r0:r0 + ROWS_PER_CORE]),
            "xtband": xtband,
            "gsr": gsr,
            "ger": ger,
            "npos": npos,
        })
    return in_maps, n_pos, W


def kernel(input, label):
    from concourse.bass_utils import run_bass_kernel_spmd

    in_maps, n_pos, W = _prep(input, label)
    if W not in _CACHE:
        _CACHE[W] = _build_nc(W)
    nc = _CACHE[W]

    res = None
    for attempt in range(4):
        try:
            res = run_bass_kernel_spmd(nc, in_maps, core_ids=list(range(NCORES)))
            break
        except Exception:
            if attempt == 3:
                raise
            import time
            time.sleep(45)  # device may need a moment to recover
    global LAST_RESULTS
    LAST_RESULTS = res
    total = 0.0
    for r in res.results:
        total += float(np.sum(r["out"], dtype=np.float64))
    return np.array(total / n_pos, dtype=np.float32)


LAST_RESULTS = None
